# revision 1
# baseline (speedup 1.0000x reference)
"""Trainium2 Bass kernel for nn_AutoAttention_Layer (sparse_attention).

Math (folded from the reference):
  qbar[b,d] = sum_f fs[f] * q[b,f,d]
  u[b,:]    = (qbar[b,:] @ W_query) @ W_key.T
  score[b,t]= sum_d k[b,t,d] * u[b,d] + bias*D
  a         = sigmoid(score); a = where(t < len[b], a, NEG_PAD); a /= 8
  a         = softmax(a, axis=t)
  out[b,:]  = (sum_t a[b,t] * k[b,t,:]) @ W_value        # note: input v is unused

Distribution: pure data parallel, batch 1024 -> 8 cores x 128 (one batch per
SBUF partition).
"""

import numpy as np

import concourse.bass as bass
from concourse import bacc
import concourse.mybir as mybir
from concourse.tile import TileContext
from concourse.bass_utils import run_bass_kernel_spmd

B, T, F, D = 1024, 200, 64, 64
NCORES = 8
BL = B // NCORES  # 128
NEG_PAD = float(np.float32(-(2.0**32) + 1.0))  # == -4294967296.0
F32 = mybir.dt.float32
BF16 = mybir.dt.bfloat16
I32 = mybir.dt.int32

TCH = 4          # t-chunks for k DMA / compute
TC = T // TCH    # 50


def _bc(ap, dims, off=0):
    """View an SBUF/DRAM AP with explicit free dims [[step, count], ...]."""
    return bass.AP(tensor=ap.tensor, offset=ap.offset + off, ap=[ap.ap[0]] + dims)


def build_nc(repeat=1, variant="full"):
    nc = bacc.Bacc()
    alu = mybir.AluOpType
    act = mybir.ActivationFunctionType

    q_d = nc.declare_dram_parameter("q", [BL, F * D], F32, isOutput=False)
    k_d = nc.declare_dram_parameter("k", [BL, T * D], F32, isOutput=False)
    kes_d = nc.declare_dram_parameter("kes", [BL, 1], I32, isOutput=False)
    fs_d = nc.declare_dram_parameter("fs", [F, 1], F32, isOutput=False)
    bias_d = nc.declare_dram_parameter("bias", [1, 1], F32, isOutput=False)
    wq_d = nc.declare_dram_parameter("wq", [D, D], F32, isOutput=False)
    wk_d = nc.declare_dram_parameter("wk", [D, D], F32, isOutput=False)
    wv_d = nc.declare_dram_parameter("wv", [D, D], F32, isOutput=False)
    out_d = nc.declare_dram_parameter("out", [BL, D], F32, isOutput=True)

    with TileContext(nc) as tc:
        with (
            tc.tile_pool(name="big", bufs=1) as big,
            tc.tile_pool(name="work", bufs=2) as work,
            tc.tile_pool(name="small", bufs=1) as small,
            tc.tile_pool(name="psum", bufs=1, space="PSUM") as psum,
        ):
            # ---- small constants -------------------------------------------------
            fs_b = small.tile([BL, F], F32)     # fs broadcast to all partitions
            nc.sync.dma_start(
                out=fs_b,
                in_=bass.AP(tensor=fs_d[:, :].tensor, offset=fs_d[:, :].offset,
                            ap=[[0, BL], [1, F]]),
            )
            bias_b = small.tile([BL, 1], F32)
            nc.sync.dma_start(
                out=bias_b,
                in_=bass.AP(tensor=bias_d[:, :].tensor, offset=bias_d[:, :].offset,
                            ap=[[0, BL], [1, 1]]),
            )
            kes_s = small.tile([BL, 1], I32)
            nc.sync.dma_start(out=kes_s, in_=kes_d[:, :])
            wq_s = small.tile([D, D], F32)
            nc.sync.dma_start(out=wq_s, in_=wq_d[:, :])
            wk_s = small.tile([D, D], F32)
            nc.sync.dma_start(out=wk_s, in_=wk_d[:, :])
            wv_s = small.tile([D, D], F32)
            nc.sync.dma_start(out=wv_s, in_=wv_d[:, :])

            # DVE copies of W so PE instructions depend on a single engine
            wq_c = small.tile([D, D], F32)
            nc.vector.tensor_copy(out=wq_c, in_=wq_s)
            wk_c = small.tile([D, D], F32)
            nc.vector.tensor_copy(out=wk_c, in_=wk_s)
            wv_c = small.tile([D, D], F32)
            nc.vector.tensor_copy(out=wv_c, in_=wv_s)

            # identity (f32) for PE transposes, built on device
            ident_i = small.tile([128, 128], I32)
            nc.gpsimd.iota(ident_i, [[1, 128]], base=0, channel_multiplier=-1)
            ident = small.tile([128, 128], F32)
            nc.vector.tensor_scalar(
                out=ident, in0=ident_i, scalar1=0, scalar2=None, op0=alu.is_equal
            )

            if variant == "compute":
                q_sC = big.tile([BL, F * D], F32, tag="q_s")
                nc.vector.memset(q_sC, 0.001)
                k_sC = big.tile([BL, T * D], F32, tag="k_s")
                nc.vector.memset(k_sC, 0.001)
            for _rep in range(repeat):
                # ---- big loads -------------------------------------------------------
                if variant == "compute":
                    q_s, k_s = q_sC, k_sC
                else:
                    q_s = big.tile([BL, F * D], F32, tag="q_s")
                    k_s = big.tile([BL, T * D], F32, tag="k_s")
                if variant != "compute":
                    for c in range(2):
                        nc.sync.dma_start(
                            out=q_s[:, c * 2048:(c + 1) * 2048],
                            in_=q_d[:, c * 2048:(c + 1) * 2048],
                        )
                    for c in range(TCH):
                        w = TC * D
                        nc.sync.dma_start(
                            out=k_s[:, c * w:(c + 1) * w],
                            in_=k_d[:, c * w:(c + 1) * w],
                        )
                if variant == "dma":
                    o_dma = small.tile([BL, D], F32, tag="o_s")
                    nc.vector.tensor_copy(out=o_dma, in_=k_s[:, :D])
                    nc.sync.dma_start(out=out_d[:, :], in_=o_dma)
                    continue

                # ---- qbar = sum_f fs[f] * q[b,f,d] ----------------------------------
                qs = work.tile([BL, F * D], F32, tag="qs")
                nc.vector.tensor_tensor(
                    out=qs,
                    in0=q_s,
                    in1=_bc(fs_b[:], [[1, F], [0, D]]),
                    op=alu.mult,
                )
                qbar = small.tile([BL, D], F32)
                nc.vector.tensor_reduce(
                    out=qbar,
                    in_=_bc(qs[:], [[1, D], [D, F]]),
                    axis=mybir.AxisListType.X,
                    op=alu.add,
                )

                # ---- u = (qbar @ Wq) @ Wk.T  (via transposed-space matmuls) ---------
                qbarT_p = psum.tile([D, BL], F32)
                nc.tensor.transpose(qbarT_p, qbar, ident)           # [64,128]
                qbarT = small.tile([D, BL], F32)
                nc.vector.tensor_copy(out=qbarT, in_=qbarT_p)

                qwT_p = psum.tile([D, BL], F32)
                nc.tensor.matmul(qwT_p, wq_c, qbarT, start=True, stop=True)  # qw^T[e,b]
                qwT = small.tile([D, BL], F32)
                nc.vector.tensor_copy(out=qwT, in_=qwT_p)

                wkT_p = psum.tile([D, D], F32)
                nc.tensor.transpose(wkT_p, wk_c, ident[:D, :D])     # W_key^T [e,d']
                wkT = small.tile([D, D], F32)
                nc.vector.tensor_copy(out=wkT, in_=wkT_p)

                uT_p = psum.tile([D, BL], F32)
                nc.tensor.matmul(uT_p, wkT, qwT, start=True, stop=True)      # u^T[d,b]
                uT = small.tile([D, BL], F32)
                nc.vector.tensor_copy(out=uT, in_=uT_p)

                u_p = psum.tile([BL, D], F32)
                nc.tensor.transpose(u_p, uT, ident[:D, :D])         # u [128,64]
                u_s = small.tile([BL, D], F32)
                nc.vector.tensor_copy(out=u_s, in_=u_p)

                # ---- score[b,t] = sum_d k*u ----------------------------------------
                score = small.tile([BL, T], F32)
                for c in range(TCH):
                    prod = work.tile([BL, TC * D], F32, tag="prod")
                    kc = bass.AP(tensor=k_s[:].tensor, offset=k_s[:].offset + c * TC * D,
                                 ap=[k_s[:].ap[0], [D, TC], [1, D]])
                    nc.vector.tensor_tensor(
                        out=prod,
                        in0=kc,
                        in1=_bc(u_s[:], [[0, TC], [1, D]]),
                        op=alu.mult,
                    )
                    nc.vector.tensor_reduce(
                        out=score[:, c * TC:(c + 1) * TC],
                        in_=_bc(prod[:], [[D, TC], [1, D]]),
                        axis=mybir.AxisListType.X,
                        op=alu.add,
                    )

                # ---- a = sigmoid(score + bias*D) ------------------------------------
                bias64 = small.tile([BL, 1], F32)
                nc.vector.tensor_scalar_mul(bias64, bias_b, float(D))
                a_sig = small.tile([BL, T], F32)
                nc.scalar.activation(a_sig, score, act.Sigmoid, bias=bias64[:], scale=1.0)

                # ---- masking --------------------------------------------------------
                iota_i = small.tile([BL, T], I32)
                nc.gpsimd.iota(iota_i, [[1, T]], base=0, channel_multiplier=0)
                iota_f = small.tile([BL, T], F32)
                nc.vector.tensor_copy(out=iota_f, in_=iota_i)
                len_f = small.tile([BL, 1], F32)
                nc.vector.tensor_copy(out=len_f, in_=kes_s)
                mask01 = small.tile([BL, T], F32)
                nc.vector.tensor_scalar(
                    out=mask01, in0=iota_f, scalar1=len_f[:], scalar2=None, op0=alu.is_lt
                )
                # am = a_sig*mask + NEG_PAD*(1-mask), each term exact in fp32
                pad_t = small.tile([BL, T], F32)
                nc.vector.tensor_scalar(
                    out=pad_t, in0=mask01, scalar1=-NEG_PAD, scalar2=NEG_PAD,
                    op0=alu.mult, op1=alu.add,
                )
                am = small.tile([BL, T], F32)
                nc.vector.scalar_tensor_tensor(
                    out=am, in0=a_sig, scalar=1.0, in1=mask01,
                    op0=alu.mult, op1=alu.mult,
                )
                nc.vector.tensor_tensor(out=am, in0=am, in1=pad_t, op=alu.add)

                # ---- softmax over t (with /8 folded into exp scale) -----------------
                mx = small.tile([BL, 1], F32)
                nc.vector.tensor_reduce(out=mx, in_=am, axis=mybir.AxisListType.X,
                                        op=alu.max)
                nmx8 = small.tile([BL, 1], F32)
                nc.vector.tensor_scalar_mul(nmx8, mx, -0.125)
                e_t = small.tile([BL, T], F32)
                se = small.tile([BL, 1], F32)
                nc.scalar.activation(e_t, am, act.Exp, bias=nmx8[:], scale=0.125,
                                     accum_out=se[:])
                rs = small.tile([BL, 1], F32)
                nc.vector.reciprocal(rs, se)

                # ---- abar[b,d] = sum_t e*k ------------------------------------------
                parts = []
                for c in range(TCH):
                    prod2 = work.tile([BL, TC * D], F32, tag="prod2")
                    kc = bass.AP(tensor=k_s[:].tensor, offset=k_s[:].offset + c * TC * D,
                                 ap=[k_s[:].ap[0], [D, TC], [1, D]])
                    ec = bass.AP(tensor=e_t[:].tensor, offset=e_t[:].offset + c * TC,
                                 ap=[e_t[:].ap[0], [1, TC], [0, D]])
                    nc.vector.tensor_tensor(out=prod2, in0=kc, in1=ec, op=alu.mult)
                    part = work.tile([BL, D], F32, tag="part")
                    nc.vector.tensor_reduce(
                        out=part,
                        in_=_bc(prod2[:], [[1, D], [D, TC]]),
                        axis=mybir.AxisListType.X,
                        op=alu.add,
                    )
                    parts.append(part)
                ab01 = work.tile([BL, D], F32, tag="ab01")
                nc.vector.tensor_tensor(out=ab01, in0=parts[0], in1=parts[1], op=alu.add)
                ab23 = work.tile([BL, D], F32, tag="ab23")
                nc.vector.tensor_tensor(out=ab23, in0=parts[2], in1=parts[3], op=alu.add)
                abar = small.tile([BL, D], F32)
                nc.vector.tensor_tensor(out=abar, in0=ab01, in1=ab23, op=alu.add)
                # normalize by 1/sum(e)
                nc.vector.tensor_scalar(
                    out=abar, in0=abar, scalar1=rs[:], scalar2=None, op0=alu.mult
                )

                # ---- out = abar @ W_value ------------------------------------------
                abarT_p = psum.tile([D, BL], F32)
                nc.tensor.transpose(abarT_p, abar, ident)
                abarT = small.tile([D, BL], F32)
                nc.vector.tensor_copy(out=abarT, in_=abarT_p)

                oT_p = psum.tile([D, BL], F32)
                nc.tensor.matmul(oT_p, wv_c, abarT, start=True, stop=True)   # out^T[e,b]
                oT = small.tile([D, BL], F32)
                nc.vector.tensor_copy(out=oT, in_=oT_p)

                o_p = psum.tile([BL, D], F32)
                nc.tensor.transpose(o_p, oT, ident[:D, :D])
                o_s = small.tile([BL, D], F32)
                nc.vector.tensor_copy(out=o_s, in_=o_p)
                nc.sync.dma_start(out=out_d[:, :], in_=o_s)

    nc.finalize()
    return nc


def build_micro(variant, repeat=1):
    nc = bacc.Bacc()
    alu = mybir.AluOpType
    out_d = nc.declare_dram_parameter("out", [BL, D], F32, isOutput=True)
    with TileContext(nc) as tc:
        with (
            tc.tile_pool(name="big", bufs=1) as big,
            tc.tile_pool(name="work", bufs=2) as work,
            tc.tile_pool(name="small", bufs=1) as small,
            tc.tile_pool(name="psum", bufs=2, space="PSUM") as psum,
        ):
            kM = big.tile([BL, T * D], F32, tag="k_s")
            nc.vector.memset(kM, 0.001)
            kMb = big.tile([BL, T * D], BF16, tag="k_bf")
            nc.vector.memset(kMb, 0.001)
            uM = small.tile([BL, D], F32)
            nc.vector.memset(uM, 0.001)
            uMb = small.tile([BL, D], BF16)
            nc.vector.memset(uMb, 0.001)
            eM = small.tile([BL, T], F32)
            nc.vector.memset(eM, 0.001)
            accM = small.tile([BL, D], F32)
            nc.vector.memset(accM, 0.0)
            identB = small.tile([128, 128], BF16)
            nc.vector.memset(identB, 0.0)    # contents irrelevant for timing
            diagM = big.tile([BL, D * 128], BF16, tag="diag")
            nc.vector.memset(diagM, 0.001)

            def consume(ap):
                nc.vector.tensor_tensor(out=accM, in0=accM, in1=ap, op=alu.add)

            for _rep in range(repeat):
                if variant == "micro_tt_f32":
                    o1 = work.tile([BL, T * D], F32, tag="mo")
                    nc.vector.tensor_tensor(
                        out=o1, in0=kM,
                        in1=_bc(uM[:], [[0, T], [1, D]]), op=alu.mult)
                    consume(o1[:, :D])
                elif variant == "micro_tt_bf16":
                    o2 = work.tile([BL, T * D], BF16, tag="mo")
                    nc.vector.tensor_tensor(
                        out=o2, in0=kMb,
                        in1=_bc(uMb[:], [[0, T], [1, D]]), op=alu.mult)
                    consume(o2[:, :D])
                elif variant == "micro_red_f32":
                    o3 = work.tile([BL, T], F32, tag="mo")
                    nc.vector.tensor_reduce(
                        out=o3, in_=_bc(kM[:], [[D, T], [1, D]]),
                        axis=mybir.AxisListType.X, op=alu.add)
                    consume(o3[:, :D])
                elif variant == "micro_red_bf16":
                    o4 = work.tile([BL, T], F32, tag="mo")
                    nc.vector.tensor_reduce(
                        out=o4, in_=_bc(kMb[:], [[D, T], [1, D]]),
                        axis=mybir.AxisListType.X, op=alu.add)
                    consume(o4[:, :D])
                elif variant == "micro_red_strided":
                    o5 = work.tile([BL, D], F32, tag="mo")
                    nc.vector.tensor_reduce(
                        out=o5, in_=_bc(kM[:], [[1, D], [D, T]]),
                        axis=mybir.AxisListType.X, op=alu.add)
                    consume(o5)
                elif variant == "micro_ttbig_bf16":
                    o6 = work.tile([BL, T * D], BF16, tag="mo")
                    nc.vector.tensor_tensor(
                        out=o6, in0=kMb,
                        in1=_bc(eM[:], [[1, T], [0, D]]), op=alu.mult)
                    consume(o6[:, :D])
                elif variant == "micro_cast":
                    o7 = work.tile([BL, T * D], BF16, tag="mo")
                    nc.scalar.copy(out=o7, in_=kM)
                    consume(o7[:, :D])
                elif variant == "micro_tree_bf16":
                    o8 = work.tile([BL, 6400], BF16, tag="mo")
                    nc.vector.tensor_tensor(
                        out=o8, in0=kMb[:, :6400], in1=kMb[:, 6400:12800],
                        op=alu.add)
                    consume(o8[:, :D])
                elif variant == "micro_diagbuild":
                    o10 = big.tile([BL, D * 128], BF16, tag="diag2")
                    nc.vector.tensor_tensor(
                        out=o10,
                        in0=_bc(identB[:], [[0, D], [1, 128]]),
                        in1=_bc(uMb[:], [[1, D], [0, 128]]),
                        op=alu.mult)
                    consume(o10[:, :D])
                elif variant == "micro_pescore_mm":
                    ps = psum.tile([BL, T], F32, tag="ps")
                    for dd in range(D):
                        nc.tensor.matmul(
                            ps,
                            _bc(diagM[:], [[1, 128]], off=dd * 128),
                            _bc(kMb[:], [[D, T]], off=dd),
                            start=(dd == 0), stop=(dd == D - 1))
                    o11 = work.tile([BL, T], F32, tag="mo")
                    nc.vector.tensor_copy(out=o11, in_=ps)
                    consume(o11[:, :D])
                else:
                    raise ValueError(variant)
            nc.sync.dma_start(out=out_d[:, :], in_=accM)
    nc.finalize()
    return nc


_NC_CACHE = {}


def _get_nc():
    if "nc" not in _NC_CACHE:
        _NC_CACHE["nc"] = build_nc()
    return _NC_CACHE["nc"]


def make_in_maps(q, k, kes_length, field_strengths, bias, W_query, W_key, W_value):
    f32 = np.float32
    maps = []
    for c in range(NCORES):
        sl = slice(c * BL, (c + 1) * BL)
        maps.append({
            "q": np.ascontiguousarray(q[sl].reshape(BL, F * D), dtype=f32),
            "k": np.ascontiguousarray(k[sl].reshape(BL, T * D), dtype=f32),
            "kes": np.ascontiguousarray(kes_length[sl].reshape(BL, 1), dtype=np.int32),
            "fs": np.ascontiguousarray(field_strengths.reshape(F, 1), dtype=f32),
            "bias": np.ascontiguousarray(bias.reshape(1, 1), dtype=f32),
            "wq": np.ascontiguousarray(W_query, dtype=f32),
            "wk": np.ascontiguousarray(W_key, dtype=f32),
            "wv": np.ascontiguousarray(W_value, dtype=f32),
        })
    return maps


def kernel(q, k, v, kes_length, field_strengths, bias, W_query, W_key, W_value,
           **_unused):
    nc = _get_nc()
    in_maps = make_in_maps(np.asarray(q), np.asarray(k), np.asarray(kes_length),
                           np.asarray(field_strengths), np.asarray(bias),
                           np.asarray(W_query), np.asarray(W_key),
                           np.asarray(W_value))
    res = run_bass_kernel_spmd(nc, in_maps, list(range(NCORES)))
    out = np.concatenate([res.results[c]["out"] for c in range(NCORES)], axis=0)
    return out.reshape(B, 1, D).astype(np.float32)



# revision 21
# speedup vs baseline: 10.3441x; 10.3441x over previous
"""Trainium2 Bass kernel for nn_AutoAttention_Layer (sparse_attention).

Math (folded from the reference):
  qbar[b,d] = sum_f fs[f] * q[b,f,d]
  u[b,:]    = (qbar[b,:] @ W_query) @ W_key.T
  score[b,t]= sum_d k[b,t,d] * u[b,d] + bias*D
  s         = sigmoid(score);  w = exp(s/8) ~= (1 + s/16)^2  (s/8 in (0,1/8))
  e         = w * mask  (mask = 1 if t < len else 1e-30; tiny keeps len=0 rows
              finite and reproduces the reference's uniform-softmax fallback)
  out[b,:]  = ((sum_t e*k) / sum_t e) @ W_value          # input v is unused

Engine split per 128-batch core (batch data parallel over 8 cores):
  PE   : score via 64 accumulating diag(u[:,d]) matmuls per t-chunk
  Pool : mask build, one diag group (affine_select)
  ACT  : k-chunk casts, diag replicas, e-broadcast replicas, sigmoid
  DVE  : qbar, u-chain PSUM hops, diag muls, exp-poly, e*k prod, tree-adds
  DMA  : constants, q, then k t-chunks; compute overlapped with k streaming
"""

import numpy as np

import concourse.bass as bass
from concourse import bacc
import concourse.mybir as mybir
from concourse.tile import TileContext
from concourse.bass_utils import run_bass_kernel_spmd

B, T, F, D = 1024, 200, 64, 64
NCORES = 8
BL = B // NCORES  # 128
F32 = mybir.dt.float32
BF16 = mybir.dt.bfloat16
I32 = mybir.dt.int32

# HW-safety knobs: exotic instructions that CoreSim accepts but may lack
# ucode/runtime support on this HW path. Flip individually to bisect.
USE_AFFINE = False        # gpsimd.affine_select for diag build
USE_POOL_COMPUTE = False  # gpsimd tensor ops beyond iota
USE_TTR = False           # DVE tensor_tensor_reduce fused op

TCS = [64, 48, 40, 32, 16]      # descending t-chunks: tail chain shrinks as
TOFF = [0, 64, 112, 152, 184]   # the serial DMA resource drains
# diag-group availability order (g2 on DVE lands first, then Pool's g0, ...)
DORDER = ([32 + i for i in range(16)] + [i for i in range(16)]
          + [16 + i for i in range(16)] + [48 + i for i in range(16)])


def _bc(ap, dims, off=0):
    """View an SBUF/DRAM AP with explicit free dims [[step, count], ...]."""
    return bass.AP(tensor=ap.tensor, offset=ap.offset + off, ap=[ap.ap[0]] + dims)


def build_nc(repeat=1, variant="full"):
    nc = bacc.Bacc()
    alu = mybir.AluOpType
    act = mybir.ActivationFunctionType

    q_d = nc.declare_dram_parameter("q", [BL, F * D], F32, isOutput=False)
    k_d = nc.declare_dram_parameter("k", [BL, T * D], F32, isOutput=False)
    kes_d = nc.declare_dram_parameter("kes", [BL, 1], I32, isOutput=False)
    fs_d = nc.declare_dram_parameter("fs", [F, 1], F32, isOutput=False)
    bias_d = nc.declare_dram_parameter("bias", [1, 1], F32, isOutput=False)
    wq_d = nc.declare_dram_parameter("wq", [D, D], F32, isOutput=False)
    wk_d = nc.declare_dram_parameter("wk", [D, D], F32, isOutput=False)
    wv_d = nc.declare_dram_parameter("wv", [D, D], F32, isOutput=False)
    out_d = nc.declare_dram_parameter("out", [BL, D], F32, isOutput=True)

    with TileContext(nc) as tc:
        with (
            tc.tile_pool(name="big", bufs=1) as big,
            tc.tile_pool(name="work", bufs=2) as work,
            tc.tile_pool(name="small", bufs=1) as small,
            tc.tile_pool(name="psum", bufs=1, space="PSUM") as psum,
            tc.tile_pool(name="psum2", bufs=2, space="PSUM") as psum2,
        ):
            # ---- input DMAs -----------------------------------------------
            # The DMA engine pool is one serial resource: order transfers by
            # when their consumers need them. fs first (gates qbar), then q,
            # then the W matrices (u-chain), then the k t-chunks.
            fs_b = small.tile([BL, F], F32)
            nc.sync.dma_start(
                out=fs_b,
                in_=bass.AP(tensor=fs_d[:, :].tensor, offset=fs_d[:, :].offset,
                            ap=[[0, BL], [1, F]]),
            )
            q_s = big.tile([BL, F * D], F32, tag="q_s")
            for c in range(4):
                nc.sync.dma_start(
                    out=q_s[:, c * 1024:(c + 1) * 1024],
                    in_=q_d[:, c * 1024:(c + 1) * 1024],
                )
            wq_s = small.tile([D, D], F32)
            nc.sync.dma_start(out=wq_s, in_=wq_d[:, :])
            wk_s = small.tile([D, D], F32)
            nc.sync.dma_start(out=wk_s, in_=wk_d[:, :])
            wv_s = small.tile([D, D], F32)
            nc.sync.dma_start(out=wv_s, in_=wv_d[:, :])
            k_s = big.tile([BL, T * D], F32, tag="k_s")
            for c in range(len(TCS)):
                lo, hi = TOFF[c] * D, (TOFF[c] + TCS[c]) * D
                nc.sync.dma_start(out=k_s[:, lo:hi], in_=k_d[:, lo:hi])

            bias_b = small.tile([BL, 1], F32)
            nc.gpsimd.dma_start(
                out=bias_b,
                in_=bass.AP(tensor=bias_d[:, :].tensor, offset=bias_d[:, :].offset,
                            ap=[[0, BL], [1, 1]]),
            )
            kes_s = small.tile([BL, 1], I32)
            nc.gpsimd.dma_start(out=kes_s, in_=kes_d[:, :])

            # ---- Pool: identities + sequence mask -------------------------
            pe_ = nc.gpsimd if USE_POOL_COMPUTE else nc.vector
            ident_i = small.tile([128, 128], I32)
            nc.gpsimd.iota(ident_i, [[1, 128]], base=0, channel_multiplier=-1)
            identf = small.tile([128, 128], F32)
            pe_.tensor_scalar(
                out=identf, in0=ident_i, scalar1=0, scalar2=None, op0=alu.is_equal
            )
            identb = small.tile([128, 128], BF16)
            pe_.tensor_scalar(
                out=identb, in0=ident_i, scalar1=0, scalar2=None, op0=alu.is_equal
            )
            iota_i = small.tile([BL, T], I32)
            nc.gpsimd.iota(iota_i, [[1, T]], base=0, channel_multiplier=0)
            iota_f = small.tile([BL, T], F32)
            pe_.tensor_copy(out=iota_f, in_=iota_i)
            len_f = small.tile([BL, 1], F32)
            pe_.tensor_copy(out=len_f, in_=kes_s)
            mask01 = small.tile([BL, T], F32)
            pe_.tensor_scalar(
                out=mask01, in0=iota_f, scalar1=len_f[:], scalar2=None, op0=alu.is_lt
            )
            # mask_bf = 1.0 where valid, 1e-30 where masked (len=0 fallback)
            mask_bf = small.tile([BL, T], BF16)
            pe_.tensor_scalar(
                out=mask_bf, in0=mask01, scalar1=1.0, scalar2=1e-30,
                op0=alu.mult, op1=alu.add,
            )

            # ACT: pin the sigmoid act-func table before any Copy activation
            # so only one table load happens (Copy is in every table).
            dum = small.tile([BL, 1], F32)
            nc.scalar.activation(dum, bias_b, act.Sigmoid, bias=0.0, scale=1.0)

            # k chunk-0 cast early on ACT (k arrives ~10us; ACT idle then)
            k_bf = big.tile([BL, T * D], BF16, tag="k_bf")
            nc.scalar.copy(out=k_bf[:, 0:TCS[0] * D], in_=k_s[:, 0:TCS[0] * D])

            # ---- qbar = sum_f fs[f]*q  (ACT casts, DVE 2x TTs + trees) ----
            fs_exp = big.tile([BL, F * D], BF16, tag="fs_exp")
            nc.scalar.copy(out=fs_exp, in_=_bc(fs_b[:], [[1, F], [0, D]]))
            prod_q = big.tile([BL, F * D], BF16, tag="prod_q")
            for c in range(4):
                o = c * 1024
                nc.vector.tensor_tensor(
                    out=prod_q[:, o:o + 1024],
                    in0=q_s[:, o:o + 1024],
                    in1=fs_exp[:, o:o + 1024],
                    op=alu.mult,
                )
            qtails = []
            for c in range(4):
                o = c * 1024
                tq1 = work.tile([BL, 512], BF16, tag="tq1")
                nc.vector.tensor_tensor(out=tq1, in0=prod_q[:, o:o + 512],
                                        in1=prod_q[:, o + 512:o + 1024], op=alu.add)
                tq2 = work.tile([BL, 256], BF16, tag="tq2")
                nc.vector.tensor_tensor(out=tq2, in0=tq1[:, :256],
                                        in1=tq1[:, 256:512], op=alu.add)
                tq3 = work.tile([BL, 128], BF16, tag="tq3")
                nc.vector.tensor_tensor(out=tq3, in0=tq2[:, :128],
                                        in1=tq2[:, 128:256], op=alu.add)
                qtails.append(tq3)
            tq4a = work.tile([BL, 128], BF16, tag="tq4")
            nc.vector.tensor_tensor(out=tq4a, in0=qtails[0], in1=qtails[1],
                                    op=alu.add)
            tq4b = work.tile([BL, 128], BF16, tag="tq4")
            nc.vector.tensor_tensor(out=tq4b, in0=qtails[2], in1=qtails[3],
                                    op=alu.add)
            tq5 = work.tile([BL, 128], BF16, tag="tq5")
            nc.vector.tensor_tensor(out=tq5, in0=tq4a, in1=tq4b, op=alu.add)
            qbar = small.tile([BL, D], F32)
            nc.vector.tensor_reduce(
                out=qbar, in_=_bc(tq5[:], [[1, D], [D, 2]]),
                axis=mybir.AxisListType.X, op=alu.add,
            )

            # ---- M = Wq @ Wk.T precomputed off the qbar critical path -----
            wqT_p = psum.tile([D, D], F32, tag="ps_wkT")
            nc.tensor.transpose(wqT_p, wq_s, identf[:D, :D])
            wqT = small.tile([D, D], F32)
            nc.vector.tensor_copy(out=wqT, in_=wqT_p)
            wkT_p = psum2.tile([D, BL], F32, tag="ps_t")
            nc.tensor.transpose(wkT_p[:, :D], wk_s, identf[:D, :D])
            wkT = small.tile([D, D], F32)
            nc.vector.tensor_copy(out=wkT, in_=wkT_p[:, :D])
            m_p = psum2.tile([D, BL], F32, tag="ps_t")
            nc.tensor.matmul(m_p[:, :D], wqT, wkT, start=True, stop=True)
            m_s = small.tile([D, D], F32)
            nc.vector.tensor_copy(out=m_s, in_=m_p[:, :D])

            # ---- u^T = M^T @ qbar^T  (3 hops after qbar) ------------------
            # (priority-pinned: this short chain gates the diag build)
            qbarT_p = psum2.tile([D, BL], F32, tag="ps_t")
            nc.tensor.transpose(qbarT_p, qbar, identf)
            qbarT = small.tile([D, BL], F32)
            nc.vector.tensor_copy(out=qbarT, in_=qbarT_p)

            u_p = psum2.tile([BL, D], F32, tag="ps_v")
            nc.tensor.matmul(u_p, qbarT, m_s, start=True, stop=True)
            u_bf = small.tile([BL, D], BF16)
            nc.vector.tensor_copy(out=u_bf, in_=u_p)

            # ---- diag(u[:,d]) weights, split across DVE/ACT/Pool ----------
            GW = 16 * 128
            diag = big.tile([BL, D * 128], BF16, tag="diag")
            with tc.high_priority():
                # g2 fully on DVE (earliest available)
                urep2 = work.tile([BL, GW], BF16, tag="urep")
                nc.vector.tensor_copy(out=urep2,
                                      in_=_bc(u_bf[:], [[1, 16], [0, 128]], off=32))
                nc.vector.tensor_tensor(
                    out=diag[:, 2 * GW:3 * GW],
                    in0=_bc(identb[:], [[0, 16], [1, 128]]),
                    in1=urep2, op=alu.mult)
                # g0 on Pool (affine) or via ACT repl + DVE mult
                if USE_AFFINE:
                    nc.gpsimd.affine_select(
                        out=diag[:, 0:GW],
                        in_=_bc(u_bf[:], [[1, 16], [0, 128]], off=0),
                        pattern=[[0, 16], [1, 128]],
                        compare_op=alu.is_equal,
                        fill=0.0,
                        base=0,
                        channel_multiplier=-1,
                    )
                    G_ACT = (1, 3)
                else:
                    G_ACT = (0, 1, 3)
                # ACT replicates, DVE multiplies by identity
                for g in G_ACT:
                    urep = work.tile([BL, GW], BF16, tag="urep")
                    nc.scalar.copy(out=urep,
                                   in_=_bc(u_bf[:], [[1, 16], [0, 128]], off=g * 16))
                    nc.vector.tensor_tensor(
                        out=diag[:, g * GW:(g + 1) * GW],
                        in0=_bc(identb[:], [[0, 16], [1, 128]]),
                        in1=urep, op=alu.mult)

            # bias*D for the sigmoid bias operand
            bias64 = small.tile([BL, 1], F32)
            nc.vector.tensor_scalar_mul(bias64, bias_b, float(D))

            # ---- per t-chunk, pass 1a: cast, PE score, sigmoid ------------
            ps_score = psum.tile([BL, T], F32, tag="ps_score")
            CAST_ENG = [None, "dve", "act", "act", "dve"]
            EEXP_ENG = ["dve", "act", "act", "act", None]
            sig_cs = []
            for c, tcsz in enumerate(TCS):
                toff = TOFF[c]
                lo, hi = toff * D, (toff + tcsz) * D
                if CAST_ENG[c] == "act":
                    nc.scalar.copy(out=k_bf[:, lo:hi], in_=k_s[:, lo:hi])
                elif CAST_ENG[c] == "dve":
                    nc.vector.tensor_copy(out=k_bf[:, lo:hi], in_=k_s[:, lo:hi])

                # PE: score[:, chunk] = sum_d diag_d @ k_bf[:, d::D]
                # (d in diag-group availability order; accumulation commutes)
                for i, d in enumerate(DORDER):
                    nc.tensor.matmul(
                        ps_score[:, toff:toff + tcsz],
                        _bc(diag[:], [[1, 128]], off=d * 128),
                        _bc(k_bf[:], [[D, tcsz]], off=lo + d),
                        start=(i == 0), stop=(i == D - 1),
                    )
                # sigmoid(score + bias*D) straight out of PSUM
                sig_c = work.tile([BL, tcsz], BF16, tag="sig")
                nc.scalar.activation(sig_c, ps_score[:, toff:toff + tcsz],
                                     act.Sigmoid, bias=bias64[:], scale=1.0)
                sig_cs.append(sig_c)

            # ---- pass 1b: w = (1+sig/16)^2, e = w*mask, se chain, e_exp ---
            se_prev = None
            e_cs, e_exps = [], []
            for c, tcsz in enumerate(TCS):
                toff = TOFF[c]
                t_c = work.tile([BL, tcsz], BF16, tag="tpoly")
                nc.vector.tensor_scalar(
                    out=t_c, in0=sig_cs[c], scalar1=1.0 / 16.0, scalar2=1.0,
                    op0=alu.mult, op1=alu.add,
                )
                tm_c = work.tile([BL, tcsz], BF16, tag="tmpoly")
                nc.vector.tensor_tensor(
                    out=tm_c, in0=t_c, in1=mask_bf[:, toff:toff + tcsz],
                    op=alu.mult,
                )
                e_c = work.tile([BL, tcsz], BF16, tag="e")
                se_c = work.tile([BL, 1], F32, tag="se")
                if USE_TTR:
                    nc.vector.tensor_tensor_reduce(
                        out=e_c, in0=t_c, in1=tm_c, scale=1.0,
                        scalar=(0.0 if se_prev is None else se_prev[:]),
                        op0=alu.mult, op1=alu.add, accum_out=se_c[:],
                    )
                else:
                    nc.vector.tensor_tensor(out=e_c, in0=t_c, in1=tm_c,
                                            op=alu.mult)
                    sp_c = work.tile([BL, 1], F32, tag="sep")
                    nc.vector.tensor_reduce(out=sp_c, in_=e_c,
                                            axis=mybir.AxisListType.X, op=alu.add)
                    if se_prev is None:
                        se_c = sp_c
                    else:
                        nc.vector.tensor_tensor(out=se_c, in0=sp_c,
                                                in1=se_prev, op=alu.add)
                se_prev = se_c
                e_cs.append(e_c)
                if EEXP_ENG[c] == "act":
                    e_exp = work.tile([BL, tcsz * D], BF16, tag="eexp")
                    nc.scalar.copy(out=e_exp,
                                   in_=_bc(e_c[:], [[1, tcsz], [0, D]]))
                elif EEXP_ENG[c] == "dve":
                    e_exp = work.tile([BL, tcsz * D], BF16, tag="eexp")
                    nc.vector.tensor_copy(out=e_exp,
                                          in_=_bc(e_c[:], [[1, tcsz], [0, D]]))
                else:
                    e_exp = None
                e_exps.append(e_exp)

            # ---- per t-chunk, pass 2: prod = k*e, halving tree over t -----
            abar_parts = []
            for c, tcsz in enumerate(TCS):
                toff = TOFF[c]
                lo, hi = toff * D, (toff + tcsz) * D
                prod_c = work.tile([BL, tcsz * D], BF16, tag="prod")
                if e_exps[c] is not None:
                    nc.vector.tensor_tensor(
                        out=prod_c, in0=k_bf[:, lo:hi], in1=e_exps[c],
                        op=alu.mult,
                    )
                else:
                    nc.vector.tensor_tensor(
                        out=prod_c, in0=k_bf[:, lo:hi],
                        in1=_bc(e_cs[c][:], [[1, tcsz], [0, D]]), op=alu.mult,
                    )
                # halving tree inside one scratch tile (levels at offsets)
                scratch = work.tile([BL, tcsz * D], BF16, tag="tree")
                cur, off, w = prod_c, 0, tcsz * D
                while w > 2 * D and (w // D) % 2 == 0:
                    w //= 2
                    nxt = scratch[:, off:off + w]
                    eng = (nc.gpsimd if (USE_POOL_COMPUTE and w <= 512 and c < 3)
                           else nc.vector)
                    eng.tensor_tensor(out=nxt, in0=cur[:, :w],
                                      in1=cur[:, w:2 * w], op=alu.add)
                    cur = nxt
                    off += w
                abar_c = work.tile([BL, D], F32, tag="abar_c")
                if w == 2 * D:
                    nc.vector.tensor_tensor(out=abar_c, in0=cur[:, :D],
                                            in1=cur[:, D:2 * D], op=alu.add)
                else:
                    nc.vector.tensor_reduce(
                        out=abar_c, in_=_bc(cur[:], [[1, D], [D, w // D]]),
                        axis=mybir.AxisListType.X, op=alu.add,
                    )
                abar_parts.append(abar_c)

            ab01 = work.tile([BL, D], F32, tag="ab01")
            nc.vector.tensor_tensor(out=ab01, in0=abar_parts[0],
                                    in1=abar_parts[1], op=alu.add)
            ab23 = work.tile([BL, D], F32, tag="ab23")
            nc.vector.tensor_tensor(out=ab23, in0=abar_parts[2],
                                    in1=abar_parts[3], op=alu.add)
            ab014 = work.tile([BL, D], F32, tag="ab014")
            nc.vector.tensor_tensor(out=ab014, in0=ab01,
                                    in1=abar_parts[4], op=alu.add)
            rs = small.tile([BL, 1], F32)
            nc.vector.reciprocal(rs, se_prev)
            abar = small.tile([BL, D], F32)
            nc.vector.tensor_tensor(out=abar, in0=ab014, in1=ab23, op=alu.add)
            nc.vector.tensor_scalar(
                out=abar, in0=abar, scalar1=rs[:], scalar2=None, op0=alu.mult
            )

            # ---- out = abar @ W_value  (f32 PE chain) ---------------------
            abarT_p = psum2.tile([D, BL], F32, tag="ps_t")
            nc.tensor.transpose(abarT_p, abar, identf)
            abarT = small.tile([D, BL], F32)
            nc.scalar.copy(out=abarT, in_=abarT_p)

            o_p = psum2.tile([BL, D], F32, tag="ps_v")
            nc.tensor.matmul(o_p, abarT, wv_s, start=True, stop=True)
            o_s = small.tile([BL, D], F32)
            nc.scalar.copy(out=o_s, in_=o_p)
            nc.sync.dma_start(out=out_d[:, :], in_=o_s)

    nc.finalize()
    return nc


_NC_CACHE = {}


def _get_nc():
    if "nc" not in _NC_CACHE:
        _NC_CACHE["nc"] = build_nc()
    return _NC_CACHE["nc"]


def make_in_maps(q, k, kes_length, field_strengths, bias, W_query, W_key, W_value):
    f32 = np.float32
    maps = []
    for c in range(NCORES):
        sl = slice(c * BL, (c + 1) * BL)
        maps.append({
            "q": np.ascontiguousarray(q[sl].reshape(BL, F * D), dtype=f32),
            "k": np.ascontiguousarray(k[sl].reshape(BL, T * D), dtype=f32),
            "kes": np.ascontiguousarray(kes_length[sl].reshape(BL, 1), dtype=np.int32),
            "fs": np.ascontiguousarray(field_strengths.reshape(F, 1), dtype=f32),
            "bias": np.ascontiguousarray(bias.reshape(1, 1), dtype=f32),
            "wq": np.ascontiguousarray(W_query, dtype=f32),
            "wk": np.ascontiguousarray(W_key, dtype=f32),
            "wv": np.ascontiguousarray(W_value, dtype=f32),
        })
    return maps


def kernel(q, k, v, kes_length, field_strengths, bias, W_query, W_key, W_value,
           **_unused):
    nc = _get_nc()
    in_maps = make_in_maps(np.asarray(q), np.asarray(k), np.asarray(kes_length),
                           np.asarray(field_strengths), np.asarray(bias),
                           np.asarray(W_query), np.asarray(W_key),
                           np.asarray(W_value))
    res = run_bass_kernel_spmd(nc, in_maps, list(range(NCORES)))
    out = np.concatenate([res.results[c]["out"] for c in range(NCORES)], axis=0)
    return out.reshape(B, 1, D).astype(np.float32)


# revision 23
# speedup vs baseline: 10.4122x; 1.0066x over previous
"""Trainium2 Bass kernel for nn_AutoAttention_Layer (sparse_attention).

Math (folded from the reference):
  qbar[b,d] = sum_f fs[f] * q[b,f,d]
  u[b,:]    = (qbar[b,:] @ W_query) @ W_key.T
  score[b,t]= sum_d k[b,t,d] * u[b,d] + bias*D
  s         = sigmoid(score);  w = exp(s/8) ~= (1 + s/16)^2  (s/8 in (0,1/8))
  e         = w * mask  (mask = 1 if t < len else 1e-30; tiny keeps len=0 rows
              finite and reproduces the reference's uniform-softmax fallback)
  out[b,:]  = ((sum_t e*k) / sum_t e) @ W_value          # input v is unused

Engine split per 128-batch core (batch data parallel over 8 cores):
  PE   : score via 64 accumulating diag(u[:,d]) matmuls per t-chunk
  Pool : mask build, one diag group (affine_select)
  ACT  : k-chunk casts, diag replicas, e-broadcast replicas, sigmoid
  DVE  : qbar, u-chain PSUM hops, diag muls, exp-poly, e*k prod, tree-adds
  DMA  : constants, q, then k t-chunks; compute overlapped with k streaming
"""

import numpy as np

import concourse.bass as bass
from concourse import bacc
import concourse.mybir as mybir
from concourse.tile import TileContext
from concourse.bass_utils import run_bass_kernel_spmd

B, T, F, D = 1024, 200, 64, 64
NCORES = 8
BL = B // NCORES  # 128
F32 = mybir.dt.float32
BF16 = mybir.dt.bfloat16
I32 = mybir.dt.int32

# HW-safety knobs: exotic instructions that CoreSim accepts but may lack
# ucode/runtime support on this HW path. Flip individually to bisect.
USE_AFFINE = False        # gpsimd.affine_select for diag build
USE_POOL_COMPUTE = False  # gpsimd tensor ops beyond iota
USE_TTR = False           # DVE tensor_tensor_reduce fused op

TCS = [64, 48, 40, 32, 16]      # descending t-chunks: tail chain shrinks as
TOFF = [0, 64, 112, 152, 184]   # the serial DMA resource drains
# diag-group availability order (g2 on DVE lands first, then Pool's g0, ...)
DORDER = ([32 + i for i in range(16)] + [i for i in range(16)]
          + [16 + i for i in range(16)] + [48 + i for i in range(16)])


def _bc(ap, dims, off=0):
    """View an SBUF/DRAM AP with explicit free dims [[step, count], ...]."""
    return bass.AP(tensor=ap.tensor, offset=ap.offset + off, ap=[ap.ap[0]] + dims)


def build_nc(repeat=1, variant="full"):
    nc = bacc.Bacc()
    alu = mybir.AluOpType
    act = mybir.ActivationFunctionType

    q_d = nc.declare_dram_parameter("q", [BL, F * D], F32, isOutput=False)
    k_d = nc.declare_dram_parameter("k", [BL, T * D], F32, isOutput=False)
    kes_d = nc.declare_dram_parameter("kes", [BL, 1], I32, isOutput=False)
    fs_d = nc.declare_dram_parameter("fs", [F, 1], F32, isOutput=False)
    bias_d = nc.declare_dram_parameter("bias", [1, 1], F32, isOutput=False)
    wq_d = nc.declare_dram_parameter("wq", [D, D], F32, isOutput=False)
    wk_d = nc.declare_dram_parameter("wk", [D, D], F32, isOutput=False)
    wv_d = nc.declare_dram_parameter("wv", [D, D], F32, isOutput=False)
    out_d = nc.declare_dram_parameter("out", [BL, D], F32, isOutput=True)

    with TileContext(nc) as tc:
        with (
            tc.tile_pool(name="big", bufs=1) as big,
            tc.tile_pool(name="work", bufs=2) as work,
            tc.tile_pool(name="small", bufs=1) as small,
            tc.tile_pool(name="psum", bufs=1, space="PSUM") as psum,
            tc.tile_pool(name="psum2", bufs=2, space="PSUM") as psum2,
        ):
            # ---- input DMAs -----------------------------------------------
            # The DMA engine pool is one serial resource: order transfers by
            # when their consumers need them. fs first (gates qbar), then q,
            # then the W matrices (u-chain), then the k t-chunks.
            fs_b = small.tile([BL, F], F32)
            nc.sync.dma_start(
                out=fs_b,
                in_=bass.AP(tensor=fs_d[:, :].tensor, offset=fs_d[:, :].offset,
                            ap=[[0, BL], [1, F]]),
            )
            q_s = big.tile([BL, F * D], F32, tag="q_s")
            for c in range(4):
                nc.sync.dma_start(
                    out=q_s[:, c * 1024:(c + 1) * 1024],
                    in_=q_d[:, c * 1024:(c + 1) * 1024],
                )
            wq_s = small.tile([D, D], F32)
            nc.sync.dma_start(out=wq_s, in_=wq_d[:, :])
            wk_s = small.tile([D, D], F32)
            nc.sync.dma_start(out=wk_s, in_=wk_d[:, :])
            wv_s = small.tile([D, D], F32)
            nc.sync.dma_start(out=wv_s, in_=wv_d[:, :])
            k_s = big.tile([BL, T * D], F32, tag="k_s")
            for c in range(len(TCS)):
                lo, hi = TOFF[c] * D, (TOFF[c] + TCS[c]) * D
                nc.sync.dma_start(out=k_s[:, lo:hi], in_=k_d[:, lo:hi])

            bias_b = small.tile([BL, 1], F32)
            nc.gpsimd.dma_start(
                out=bias_b,
                in_=bass.AP(tensor=bias_d[:, :].tensor, offset=bias_d[:, :].offset,
                            ap=[[0, BL], [1, 1]]),
            )
            kes_s = small.tile([BL, 1], I32)
            nc.gpsimd.dma_start(out=kes_s, in_=kes_d[:, :])

            # ---- Pool: identities + sequence mask -------------------------
            pe_ = nc.gpsimd if USE_POOL_COMPUTE else nc.vector
            ident_i = small.tile([128, 128], I32)
            nc.gpsimd.iota(ident_i, [[1, 128]], base=0, channel_multiplier=-1)
            identf = small.tile([128, 128], F32)
            pe_.tensor_scalar(
                out=identf, in0=ident_i, scalar1=0, scalar2=None, op0=alu.is_equal
            )
            identb = small.tile([128, 128], BF16)
            pe_.tensor_scalar(
                out=identb, in0=ident_i, scalar1=0, scalar2=None, op0=alu.is_equal
            )
            iota_i = small.tile([BL, T], I32)
            nc.gpsimd.iota(iota_i, [[1, T]], base=0, channel_multiplier=0)
            iota_f = small.tile([BL, T], F32)
            pe_.tensor_copy(out=iota_f, in_=iota_i)
            len_f = small.tile([BL, 1], F32)
            pe_.tensor_copy(out=len_f, in_=kes_s)
            mask01 = small.tile([BL, T], F32)
            pe_.tensor_scalar(
                out=mask01, in0=iota_f, scalar1=len_f[:], scalar2=None, op0=alu.is_lt
            )
            # mask_bf = 1.0 where valid, 1e-30 where masked (len=0 fallback)
            mask_bf = small.tile([BL, T], BF16)
            pe_.tensor_scalar(
                out=mask_bf, in0=mask01, scalar1=1.0, scalar2=1e-30,
                op0=alu.mult, op1=alu.add,
            )

            # ACT: pin the sigmoid act-func table before any Copy activation
            # so only one table load happens (Copy is in every table).
            dum = small.tile([BL, 1], F32)
            nc.scalar.activation(dum, bias_b, act.Sigmoid, bias=0.0, scale=1.0)

            # k chunk-0 cast early on ACT (k arrives ~10us; ACT idle then)
            k_bf = big.tile([BL, T * D], BF16, tag="k_bf")
            nc.scalar.copy(out=k_bf[:, 0:TCS[0] * D], in_=k_s[:, 0:TCS[0] * D])

            # ---- qbar = sum_f fs[f]*q  (ACT casts, DVE 2x TTs + trees) ----
            fs_exp = big.tile([BL, F * D], BF16, tag="fs_exp")
            nc.scalar.copy(out=fs_exp, in_=_bc(fs_b[:], [[1, F], [0, D]]))
            prod_q = big.tile([BL, F * D], BF16, tag="prod_q")
            for c in range(4):
                o = c * 1024
                nc.vector.tensor_tensor(
                    out=prod_q[:, o:o + 1024],
                    in0=q_s[:, o:o + 1024],
                    in1=fs_exp[:, o:o + 1024],
                    op=alu.mult,
                )
            qtails = []
            for c in range(4):
                o = c * 1024
                tq1 = work.tile([BL, 512], BF16, tag="tq1")
                nc.vector.tensor_tensor(out=tq1, in0=prod_q[:, o:o + 512],
                                        in1=prod_q[:, o + 512:o + 1024], op=alu.add)
                tq2 = work.tile([BL, 256], BF16, tag="tq2")
                nc.vector.tensor_tensor(out=tq2, in0=tq1[:, :256],
                                        in1=tq1[:, 256:512], op=alu.add)
                tq3 = work.tile([BL, 128], BF16, tag="tq3")
                nc.vector.tensor_tensor(out=tq3, in0=tq2[:, :128],
                                        in1=tq2[:, 128:256], op=alu.add)
                qtails.append(tq3)
            tq4a = work.tile([BL, 128], BF16, tag="tq4")
            nc.vector.tensor_tensor(out=tq4a, in0=qtails[0], in1=qtails[1],
                                    op=alu.add)
            tq4b = work.tile([BL, 128], BF16, tag="tq4")
            nc.vector.tensor_tensor(out=tq4b, in0=qtails[2], in1=qtails[3],
                                    op=alu.add)
            tq5 = work.tile([BL, 128], BF16, tag="tq5")
            nc.vector.tensor_tensor(out=tq5, in0=tq4a, in1=tq4b, op=alu.add)
            qbar = small.tile([BL, D], F32)
            nc.vector.tensor_reduce(
                out=qbar, in_=_bc(tq5[:], [[1, D], [D, 2]]),
                axis=mybir.AxisListType.X, op=alu.add,
            )

            # ---- M = Wq @ Wk.T precomputed off the qbar critical path -----
            wqT_p = psum.tile([D, D], F32, tag="ps_wkT")
            nc.tensor.transpose(wqT_p, wq_s, identf[:D, :D])
            wqT = small.tile([D, D], F32)
            nc.vector.tensor_copy(out=wqT, in_=wqT_p)
            wkT_p = psum2.tile([D, BL], F32, tag="ps_t")
            nc.tensor.transpose(wkT_p[:, :D], wk_s, identf[:D, :D])
            wkT = small.tile([D, D], F32)
            nc.vector.tensor_copy(out=wkT, in_=wkT_p[:, :D])
            m_p = psum2.tile([D, BL], F32, tag="ps_t")
            nc.tensor.matmul(m_p[:, :D], wqT, wkT, start=True, stop=True)
            m_s = small.tile([D, D], F32)
            nc.vector.tensor_copy(out=m_s, in_=m_p[:, :D])

            # ---- u^T = M^T @ qbar^T  (3 hops after qbar) ------------------
            # (priority-pinned: this short chain gates the diag build)
            qbarT_p = psum2.tile([D, BL], F32, tag="ps_t")
            nc.tensor.transpose(qbarT_p, qbar, identf)
            qbarT = small.tile([D, BL], F32)
            nc.vector.tensor_copy(out=qbarT, in_=qbarT_p)

            u_p = psum2.tile([BL, D], F32, tag="ps_v")
            nc.tensor.matmul(u_p, qbarT, m_s, start=True, stop=True)
            u_bf = small.tile([BL, D], BF16)
            nc.vector.tensor_copy(out=u_bf, in_=u_p)

            # ---- diag(u[:,d]) weights, split across DVE/ACT/Pool ----------
            GW = 16 * 128
            diag = big.tile([BL, D * 128], BF16, tag="diag")
            with tc.high_priority():
                # g2 fully on DVE (earliest available)
                urep2 = work.tile([BL, GW], BF16, tag="urep")
                nc.vector.tensor_copy(out=urep2,
                                      in_=_bc(u_bf[:], [[1, 16], [0, 128]], off=32))
                nc.vector.tensor_tensor(
                    out=diag[:, 2 * GW:3 * GW],
                    in0=_bc(identb[:], [[0, 16], [1, 128]]),
                    in1=urep2, op=alu.mult)
                # g0 on Pool (affine) or via ACT repl + DVE mult
                if USE_AFFINE:
                    nc.gpsimd.affine_select(
                        out=diag[:, 0:GW],
                        in_=_bc(u_bf[:], [[1, 16], [0, 128]], off=0),
                        pattern=[[0, 16], [1, 128]],
                        compare_op=alu.is_equal,
                        fill=0.0,
                        base=0,
                        channel_multiplier=-1,
                    )
                    G_ACT = (1, 3)
                else:
                    G_ACT = (0, 1, 3)
                # ACT replicates, DVE multiplies by identity
                for g in G_ACT:
                    urep = work.tile([BL, GW], BF16, tag="urep")
                    nc.scalar.copy(out=urep,
                                   in_=_bc(u_bf[:], [[1, 16], [0, 128]], off=g * 16))
                    nc.vector.tensor_tensor(
                        out=diag[:, g * GW:(g + 1) * GW],
                        in0=_bc(identb[:], [[0, 16], [1, 128]]),
                        in1=urep, op=alu.mult)

            # bias*D for the sigmoid bias operand
            bias64 = small.tile([BL, 1], F32)
            nc.vector.tensor_scalar_mul(bias64, bias_b, float(D))

            # ---- per t-chunk, pass 1a: cast, PE score, sigmoid ------------
            ps_score = psum.tile([BL, T], F32, tag="ps_score")
            CAST_ENG = [None, "dve", "act", "act", "dve"]
            EEXP_ENG = ["dve", "act", "act", "act", None]
            sig_cs = []
            for c, tcsz in enumerate(TCS):
                toff = TOFF[c]
                lo, hi = toff * D, (toff + tcsz) * D
                if CAST_ENG[c] == "act":
                    nc.scalar.copy(out=k_bf[:, lo:hi], in_=k_s[:, lo:hi])
                elif CAST_ENG[c] == "dve":
                    nc.vector.tensor_copy(out=k_bf[:, lo:hi], in_=k_s[:, lo:hi])

                # PE: score[:, chunk] = sum_d diag_d @ k_bf[:, d::D]
                # (d in diag-group availability order; accumulation commutes)
                for i, d in enumerate(DORDER):
                    nc.tensor.matmul(
                        ps_score[:, toff:toff + tcsz],
                        _bc(diag[:], [[1, 128]], off=d * 128),
                        _bc(k_bf[:], [[D, tcsz]], off=lo + d),
                        start=(i == 0), stop=(i == D - 1),
                    )
                # sigmoid(score + bias*D) straight out of PSUM
                sig_c = work.tile([BL, tcsz], BF16, tag="sig")
                nc.scalar.activation(sig_c, ps_score[:, toff:toff + tcsz],
                                     act.Sigmoid, bias=bias64[:], scale=1.0)
                sig_cs.append(sig_c)

            # ---- pass 1b: w = (1+sig/16)^2, e = w*mask, se chain, e_exp ---
            se_prev = None
            e_cs, e_exps = [], []
            for c, tcsz in enumerate(TCS):
                toff = TOFF[c]
                t_c = work.tile([BL, tcsz], BF16, tag="tpoly")
                nc.vector.tensor_scalar(
                    out=t_c, in0=sig_cs[c], scalar1=1.0 / 16.0, scalar2=1.0,
                    op0=alu.mult, op1=alu.add,
                )
                tm_c = work.tile([BL, tcsz], BF16, tag="tmpoly")
                nc.vector.tensor_tensor(
                    out=tm_c, in0=t_c, in1=mask_bf[:, toff:toff + tcsz],
                    op=alu.mult,
                )
                e_c = work.tile([BL, tcsz], BF16, tag="e")
                se_c = work.tile([BL, 1], F32, tag="se")
                if USE_TTR:
                    nc.vector.tensor_tensor_reduce(
                        out=e_c, in0=t_c, in1=tm_c, scale=1.0,
                        scalar=(0.0 if se_prev is None else se_prev[:]),
                        op0=alu.mult, op1=alu.add, accum_out=se_c[:],
                    )
                else:
                    nc.vector.tensor_tensor(out=e_c, in0=t_c, in1=tm_c,
                                            op=alu.mult)
                    sp_c = work.tile([BL, 1], F32, tag="sep")
                    nc.vector.tensor_reduce(out=sp_c, in_=e_c,
                                            axis=mybir.AxisListType.X, op=alu.add)
                    if se_prev is None:
                        se_c = sp_c
                    else:
                        nc.vector.tensor_tensor(out=se_c, in0=sp_c,
                                                in1=se_prev, op=alu.add)
                se_prev = se_c
                e_cs.append(e_c)
                if EEXP_ENG[c] == "act":
                    e_exp = work.tile([BL, tcsz * D], BF16, tag="eexp")
                    nc.scalar.copy(out=e_exp,
                                   in_=_bc(e_c[:], [[1, tcsz], [0, D]]))
                elif EEXP_ENG[c] == "dve":
                    e_exp = work.tile([BL, tcsz * D], BF16, tag="eexp")
                    nc.vector.tensor_copy(out=e_exp,
                                          in_=_bc(e_c[:], [[1, tcsz], [0, D]]))
                else:
                    e_exp = None
                e_exps.append(e_exp)

            # ---- per t-chunk, pass 2: prod = k*e, halving tree over t -----
            abar_parts = []
            for c, tcsz in enumerate(TCS):
                toff = TOFF[c]
                lo, hi = toff * D, (toff + tcsz) * D
                prod_c = work.tile([BL, tcsz * D], BF16, tag="prod")
                if e_exps[c] is not None:
                    nc.vector.tensor_tensor(
                        out=prod_c, in0=k_bf[:, lo:hi], in1=e_exps[c],
                        op=alu.mult,
                    )
                else:
                    nc.vector.tensor_tensor(
                        out=prod_c, in0=k_bf[:, lo:hi],
                        in1=_bc(e_cs[c][:], [[1, tcsz], [0, D]]), op=alu.mult,
                    )
                # halving tree inside one scratch tile (levels at offsets)
                scratch = work.tile([BL, tcsz * D], BF16, tag="tree")
                cur, off, w = prod_c, 0, tcsz * D
                while w > 2 * D and (w // D) % 2 == 0:
                    w //= 2
                    nxt = scratch[:, off:off + w]
                    eng = (nc.gpsimd if (USE_POOL_COMPUTE and w <= 512 and c < 3)
                           else nc.vector)
                    eng.tensor_tensor(out=nxt, in0=cur[:, :w],
                                      in1=cur[:, w:2 * w], op=alu.add)
                    cur = nxt
                    off += w
                abar_c = work.tile([BL, D], F32, tag="abar_c")
                if w == 2 * D:
                    nc.vector.tensor_tensor(out=abar_c, in0=cur[:, :D],
                                            in1=cur[:, D:2 * D], op=alu.add)
                else:
                    nc.vector.tensor_reduce(
                        out=abar_c, in_=_bc(cur[:], [[1, D], [D, w // D]]),
                        axis=mybir.AxisListType.X, op=alu.add,
                    )
                abar_parts.append(abar_c)

            ab01 = work.tile([BL, D], F32, tag="ab01")
            nc.vector.tensor_tensor(out=ab01, in0=abar_parts[0],
                                    in1=abar_parts[1], op=alu.add)
            ab23 = work.tile([BL, D], F32, tag="ab23")
            nc.vector.tensor_tensor(out=ab23, in0=abar_parts[2],
                                    in1=abar_parts[3], op=alu.add)
            ab014 = work.tile([BL, D], F32, tag="ab014")
            nc.vector.tensor_tensor(out=ab014, in0=ab01,
                                    in1=abar_parts[4], op=alu.add)
            rs = small.tile([BL, 1], F32)
            nc.vector.reciprocal(rs, se_prev)
            abar = small.tile([BL, D], F32)
            nc.vector.tensor_tensor(out=abar, in0=ab014, in1=ab23, op=alu.add)
            nc.vector.tensor_scalar(
                out=abar, in0=abar, scalar1=rs[:], scalar2=None, op0=alu.mult
            )

            # ---- out = abar @ W_value  (f32 PE chain) ---------------------
            abarT_p = psum2.tile([D, BL], F32, tag="ps_t")
            nc.tensor.transpose(abarT_p, abar, identf)
            abarT = small.tile([D, BL], F32)
            nc.scalar.copy(out=abarT, in_=abarT_p)

            o_p = psum2.tile([BL, D], F32, tag="ps_v")
            nc.tensor.matmul(o_p, abarT, wv_s, start=True, stop=True)
            o_s = small.tile([BL, D], F32)
            nc.scalar.copy(out=o_s, in_=o_p)
            nc.sync.dma_start(out=out_d[:, :], in_=o_s)

    nc.finalize()
    return nc


_NC_CACHE = {}


def _get_nc():
    if "nc" not in _NC_CACHE:
        _NC_CACHE["nc"] = build_nc()
    return _NC_CACHE["nc"]


def make_in_maps(q, k, kes_length, field_strengths, bias, W_query, W_key, W_value):
    f32 = np.float32
    maps = []
    for c in range(NCORES):
        sl = slice(c * BL, (c + 1) * BL)
        maps.append({
            "q": np.ascontiguousarray(q[sl].reshape(BL, F * D), dtype=f32),
            "k": np.ascontiguousarray(k[sl].reshape(BL, T * D), dtype=f32),
            "kes": np.ascontiguousarray(kes_length[sl].reshape(BL, 1), dtype=np.int32),
            "fs": np.ascontiguousarray(field_strengths.reshape(F, 1), dtype=f32),
            "bias": np.ascontiguousarray(bias.reshape(1, 1), dtype=f32),
            "wq": np.ascontiguousarray(W_query, dtype=f32),
            "wk": np.ascontiguousarray(W_key, dtype=f32),
            "wv": np.ascontiguousarray(W_value, dtype=f32),
        })
    return maps


def kernel(q, k, v, kes_length, field_strengths, bias, W_query, W_key, W_value,
           **_unused):
    nc = _get_nc()
    in_maps = make_in_maps(np.asarray(q), np.asarray(k), np.asarray(kes_length),
                           np.asarray(field_strengths), np.asarray(bias),
                           np.asarray(W_query), np.asarray(W_key),
                           np.asarray(W_value))
    res = run_bass_kernel_spmd(nc, in_maps, list(range(NCORES)))
    out = np.concatenate([res.results[c]["out"] for c in range(NCORES)], axis=0)
    return out.reshape(B, 1, D).astype(np.float32)


# revision 25
# speedup vs baseline: 10.5701x; 1.0152x over previous
"""Trainium2 Bass kernel for nn_AutoAttention_Layer (sparse_attention).

Math (folded from the reference):
  qbar[b,d] = sum_f fs[f] * q[b,f,d]
  u[b,:]    = (qbar[b,:] @ W_query) @ W_key.T
  score[b,t]= sum_d k[b,t,d] * u[b,d] + bias*D
  s         = sigmoid(score);  w = exp(s/8) ~= (1 + s/16)^2  (s/8 in (0,1/8))
  e         = w * mask  (mask = 1 if t < len else 1e-30; tiny keeps len=0 rows
              finite and reproduces the reference's uniform-softmax fallback)
  out[b,:]  = ((sum_t e*k) / sum_t e) @ W_value          # input v is unused

Engine split per 128-batch core (batch data parallel over 8 cores):
  PE   : score via 64 accumulating diag(u[:,d]) matmuls per t-chunk
  Pool : mask build, one diag group (affine_select)
  ACT  : k-chunk casts, diag replicas, e-broadcast replicas, sigmoid
  DVE  : qbar, u-chain PSUM hops, diag muls, exp-poly, e*k prod, tree-adds
  DMA  : constants, q, then k t-chunks; compute overlapped with k streaming
"""

import numpy as np

import concourse.bass as bass
from concourse import bacc
import concourse.mybir as mybir
from concourse.tile import TileContext
from concourse.bass_utils import run_bass_kernel_spmd

B, T, F, D = 1024, 200, 64, 64
NCORES = 8
BL = B // NCORES  # 128
F32 = mybir.dt.float32
BF16 = mybir.dt.bfloat16
I32 = mybir.dt.int32

# HW-safety knobs: exotic instructions that CoreSim accepts but may lack
# ucode/runtime support on this HW path. Flip individually to bisect.
USE_AFFINE = False        # gpsimd.affine_select for diag build
USE_POOL_COMPUTE = False  # gpsimd tensor ops beyond iota
USE_TTR = False           # DVE tensor_tensor_reduce fused op

TCS = [64, 48, 40, 32, 16]      # descending t-chunks: tail chain shrinks as
TOFF = [0, 64, 112, 152, 184]   # the serial DMA resource drains
# diag-group availability order (g2 on DVE lands first, then Pool's g0, ...)
DORDER = ([32 + i for i in range(16)] + [i for i in range(16)]
          + [16 + i for i in range(16)] + [48 + i for i in range(16)])


def _bc(ap, dims, off=0):
    """View an SBUF/DRAM AP with explicit free dims [[step, count], ...]."""
    return bass.AP(tensor=ap.tensor, offset=ap.offset + off, ap=[ap.ap[0]] + dims)


def build_nc(repeat=1, variant="full"):
    nc = bacc.Bacc()
    alu = mybir.AluOpType
    act = mybir.ActivationFunctionType

    q_d = nc.declare_dram_parameter("q", [BL, F * D], F32, isOutput=False)
    k_d = nc.declare_dram_parameter("k", [BL, T * D], F32, isOutput=False)
    kes_d = nc.declare_dram_parameter("kes", [BL, 1], I32, isOutput=False)
    fs_d = nc.declare_dram_parameter("fs", [F, 1], F32, isOutput=False)
    bias_d = nc.declare_dram_parameter("bias", [1, 1], F32, isOutput=False)
    wq_d = nc.declare_dram_parameter("wq", [D, D], F32, isOutput=False)
    wk_d = nc.declare_dram_parameter("wk", [D, D], F32, isOutput=False)
    wv_d = nc.declare_dram_parameter("wv", [D, D], F32, isOutput=False)
    out_d = nc.declare_dram_parameter("out", [BL, D], F32, isOutput=True)

    with TileContext(nc) as tc:
        with (
            tc.tile_pool(name="big", bufs=1) as big,
            tc.tile_pool(name="work", bufs=2) as work,
            tc.tile_pool(name="small", bufs=1) as small,
            tc.tile_pool(name="psum", bufs=1, space="PSUM") as psum,
            tc.tile_pool(name="psum2", bufs=2, space="PSUM") as psum2,
        ):
            # ---- input DMAs -----------------------------------------------
            # The DMA engine pool is one serial resource: order transfers by
            # when their consumers need them. fs first (gates qbar), then q,
            # then the W matrices (u-chain), then the k t-chunks.
            fs_b = small.tile([BL, F], F32)
            nc.sync.dma_start(
                out=fs_b,
                in_=bass.AP(tensor=fs_d[:, :].tensor, offset=fs_d[:, :].offset,
                            ap=[[0, BL], [1, F]]),
            )
            q_s = big.tile([BL, F * D], F32, tag="q_s")
            for c in range(4):
                nc.sync.dma_start(
                    out=q_s[:, c * 1024:(c + 1) * 1024],
                    in_=q_d[:, c * 1024:(c + 1) * 1024],
                )
            wq_s = small.tile([D, D], F32)
            nc.sync.dma_start(out=wq_s, in_=wq_d[:, :])
            wk_s = small.tile([D, D], F32)
            nc.sync.dma_start(out=wk_s, in_=wk_d[:, :])
            wv_s = small.tile([D, D], F32)
            nc.sync.dma_start(out=wv_s, in_=wv_d[:, :])
            k_s = big.tile([BL, T * D], F32, tag="k_s")
            for c in range(len(TCS)):
                lo, hi = TOFF[c] * D, (TOFF[c] + TCS[c]) * D
                nc.sync.dma_start(out=k_s[:, lo:hi], in_=k_d[:, lo:hi])

            bias_b = small.tile([BL, 1], F32)
            nc.gpsimd.dma_start(
                out=bias_b,
                in_=bass.AP(tensor=bias_d[:, :].tensor, offset=bias_d[:, :].offset,
                            ap=[[0, BL], [1, 1]]),
            )
            kes_s = small.tile([BL, 1], I32)
            nc.gpsimd.dma_start(out=kes_s, in_=kes_d[:, :])

            # ---- Pool: identities + sequence mask -------------------------
            pe_ = nc.gpsimd if USE_POOL_COMPUTE else nc.vector
            ident_i = small.tile([128, 128], I32)
            nc.gpsimd.iota(ident_i, [[1, 128]], base=0, channel_multiplier=-1)
            identf = small.tile([128, 128], F32)
            pe_.tensor_scalar(
                out=identf, in0=ident_i, scalar1=0, scalar2=None, op0=alu.is_equal
            )
            identb = small.tile([128, 128], BF16)
            pe_.tensor_scalar(
                out=identb, in0=ident_i, scalar1=0, scalar2=None, op0=alu.is_equal
            )
            iota_i = small.tile([BL, T], I32)
            nc.gpsimd.iota(iota_i, [[1, T]], base=0, channel_multiplier=0)
            iota_f = small.tile([BL, T], F32)
            pe_.tensor_copy(out=iota_f, in_=iota_i)
            len_f = small.tile([BL, 1], F32)
            pe_.tensor_copy(out=len_f, in_=kes_s)
            mask01 = small.tile([BL, T], F32)
            pe_.tensor_scalar(
                out=mask01, in0=iota_f, scalar1=len_f[:], scalar2=None, op0=alu.is_lt
            )
            # mask_bf = 1.0 where valid, 1e-30 where masked (len=0 fallback)
            mask_bf = small.tile([BL, T], BF16)
            pe_.tensor_scalar(
                out=mask_bf, in0=mask01, scalar1=1.0, scalar2=1e-30,
                op0=alu.mult, op1=alu.add,
            )

            # ACT: pin the sigmoid act-func table before any Copy activation
            # so only one table load happens (Copy is in every table).
            dum = small.tile([BL, 1], F32)
            nc.scalar.activation(dum, bias_b, act.Sigmoid, bias=0.0, scale=1.0)

            # k chunk-0 cast early on ACT (k arrives ~10us; ACT idle then)
            k_bf = big.tile([BL, T * D], BF16, tag="k_bf")
            nc.scalar.copy(out=k_bf[:, 0:TCS[0] * D], in_=k_s[:, 0:TCS[0] * D])

            # ---- qbar = sum_f fs[f]*q  (ACT casts, DVE 2x TTs + trees) ----
            fs_exp = big.tile([BL, F * D], BF16, tag="fs_exp")
            nc.scalar.copy(out=fs_exp, in_=_bc(fs_b[:], [[1, F], [0, D]]))
            prod_q = big.tile([BL, F * D], BF16, tag="prod_q")
            for c in range(4):
                o = c * 1024
                nc.vector.tensor_tensor(
                    out=prod_q[:, o:o + 1024],
                    in0=q_s[:, o:o + 1024],
                    in1=fs_exp[:, o:o + 1024],
                    op=alu.mult,
                )
            qtails = []
            for c in range(4):
                o = c * 1024
                tq1 = work.tile([BL, 512], BF16, tag="tq1")
                nc.vector.tensor_tensor(out=tq1, in0=prod_q[:, o:o + 512],
                                        in1=prod_q[:, o + 512:o + 1024], op=alu.add)
                tq2 = work.tile([BL, 256], BF16, tag="tq2")
                nc.vector.tensor_tensor(out=tq2, in0=tq1[:, :256],
                                        in1=tq1[:, 256:512], op=alu.add)
                tq3 = work.tile([BL, 128], BF16, tag="tq3")
                nc.vector.tensor_tensor(out=tq3, in0=tq2[:, :128],
                                        in1=tq2[:, 128:256], op=alu.add)
                qtails.append(tq3)
            tq4a = work.tile([BL, 128], BF16, tag="tq4")
            nc.vector.tensor_tensor(out=tq4a, in0=qtails[0], in1=qtails[1],
                                    op=alu.add)
            tq4b = work.tile([BL, 128], BF16, tag="tq4")
            nc.vector.tensor_tensor(out=tq4b, in0=qtails[2], in1=qtails[3],
                                    op=alu.add)
            tq5 = work.tile([BL, 128], BF16, tag="tq5")
            nc.vector.tensor_tensor(out=tq5, in0=tq4a, in1=tq4b, op=alu.add)
            qbar = small.tile([BL, D], F32)
            nc.vector.tensor_reduce(
                out=qbar, in_=_bc(tq5[:], [[1, D], [D, 2]]),
                axis=mybir.AxisListType.X, op=alu.add,
            )

            # ---- M = Wq @ Wk.T precomputed off the qbar critical path -----
            wqT_p = psum.tile([D, D], F32, tag="ps_wkT")
            nc.tensor.transpose(wqT_p, wq_s, identf[:D, :D])
            wqT = small.tile([D, D], F32)
            nc.vector.tensor_copy(out=wqT, in_=wqT_p)
            wkT_p = psum2.tile([D, BL], F32, tag="ps_t")
            nc.tensor.transpose(wkT_p[:, :D], wk_s, identf[:D, :D])
            wkT = small.tile([D, D], F32)
            nc.vector.tensor_copy(out=wkT, in_=wkT_p[:, :D])
            m_p = psum2.tile([D, BL], F32, tag="ps_t")
            nc.tensor.matmul(m_p[:, :D], wqT, wkT, start=True, stop=True)
            m_s = small.tile([D, D], F32)
            nc.vector.tensor_copy(out=m_s, in_=m_p[:, :D])

            # ---- u^T = M^T @ qbar^T  (3 hops after qbar) ------------------
            # (priority-pinned: this short chain gates the diag build)
            qbarT_p = psum2.tile([D, BL], F32, tag="ps_t")
            nc.tensor.transpose(qbarT_p, qbar, identf)
            qbarT = small.tile([D, BL], F32)
            nc.vector.tensor_copy(out=qbarT, in_=qbarT_p)

            u_p = psum2.tile([BL, D], F32, tag="ps_v")
            nc.tensor.matmul(u_p, qbarT, m_s, start=True, stop=True)
            u_bf = small.tile([BL, D], BF16)
            nc.vector.tensor_copy(out=u_bf, in_=u_p)

            # ---- diag(u[:,d]) weights, split across DVE/ACT/Pool ----------
            GW = 16 * 128
            diag = big.tile([BL, D * 128], BF16, tag="diag")
            with tc.high_priority():
                # g2 fully on DVE (earliest available)
                urep2 = work.tile([BL, GW], BF16, tag="urep")
                nc.vector.tensor_copy(out=urep2,
                                      in_=_bc(u_bf[:], [[1, 16], [0, 128]], off=32))
                nc.vector.tensor_tensor(
                    out=diag[:, 2 * GW:3 * GW],
                    in0=_bc(identb[:], [[0, 16], [1, 128]]),
                    in1=urep2, op=alu.mult)
                # g0 on Pool (affine) or via ACT repl + DVE mult
                if USE_AFFINE:
                    nc.gpsimd.affine_select(
                        out=diag[:, 0:GW],
                        in_=_bc(u_bf[:], [[1, 16], [0, 128]], off=0),
                        pattern=[[0, 16], [1, 128]],
                        compare_op=alu.is_equal,
                        fill=0.0,
                        base=0,
                        channel_multiplier=-1,
                    )
                    G_ACT = (1, 3)
                else:
                    G_ACT = (0, 1, 3)
                # ACT replicates, DVE multiplies by identity
                for g in G_ACT:
                    urep = work.tile([BL, GW], BF16, tag="urep")
                    nc.scalar.copy(out=urep,
                                   in_=_bc(u_bf[:], [[1, 16], [0, 128]], off=g * 16))
                    nc.vector.tensor_tensor(
                        out=diag[:, g * GW:(g + 1) * GW],
                        in0=_bc(identb[:], [[0, 16], [1, 128]]),
                        in1=urep, op=alu.mult)

            # bias*D for the sigmoid bias operand
            bias64 = small.tile([BL, 1], F32)
            nc.vector.tensor_scalar_mul(bias64, bias_b, float(D))

            # ---- per t-chunk, pass 1a: cast, PE score, sigmoid ------------
            ps_score = psum.tile([BL, T], F32, tag="ps_score")
            CAST_ENG = [None, "dve", "act", "act", "dve"]
            EEXP_ENG = ["dve", "act", "act", "act", None]
            sig_cs = []
            for c, tcsz in enumerate(TCS):
                toff = TOFF[c]
                lo, hi = toff * D, (toff + tcsz) * D
                if CAST_ENG[c] == "act":
                    nc.scalar.copy(out=k_bf[:, lo:hi], in_=k_s[:, lo:hi])
                elif CAST_ENG[c] == "dve":
                    nc.vector.tensor_copy(out=k_bf[:, lo:hi], in_=k_s[:, lo:hi])

                # PE: score[:, chunk] = sum_d diag_d @ k_bf[:, d::D]
                # (d in diag-group availability order; accumulation commutes)
                for i, d in enumerate(DORDER):
                    nc.tensor.matmul(
                        ps_score[:, toff:toff + tcsz],
                        _bc(diag[:], [[1, 128]], off=d * 128),
                        _bc(k_bf[:], [[D, tcsz]], off=lo + d),
                        start=(i == 0), stop=(i == D - 1),
                    )
                # sigmoid(score + bias*D) straight out of PSUM
                sig_c = work.tile([BL, tcsz], BF16, tag="sig")
                nc.scalar.activation(sig_c, ps_score[:, toff:toff + tcsz],
                                     act.Sigmoid, bias=bias64[:], scale=1.0)
                sig_cs.append(sig_c)

            # ---- pass 1b: w = (1+sig/16)^2, e = w*mask, se chain, e_exp ---
            se_prev = None
            e_cs, e_exps = [], []
            for c, tcsz in enumerate(TCS):
                toff = TOFF[c]
                t_c = work.tile([BL, tcsz], BF16, tag="tpoly")
                nc.vector.tensor_scalar(
                    out=t_c, in0=sig_cs[c], scalar1=1.0 / 16.0, scalar2=1.0,
                    op0=alu.mult, op1=alu.add,
                )
                tm_c = work.tile([BL, tcsz], BF16, tag="tmpoly")
                nc.vector.tensor_tensor(
                    out=tm_c, in0=t_c, in1=mask_bf[:, toff:toff + tcsz],
                    op=alu.mult,
                )
                e_c = work.tile([BL, tcsz], BF16, tag="e")
                se_c = work.tile([BL, 1], F32, tag="se")
                if USE_TTR:
                    nc.vector.tensor_tensor_reduce(
                        out=e_c, in0=t_c, in1=tm_c, scale=1.0,
                        scalar=(0.0 if se_prev is None else se_prev[:]),
                        op0=alu.mult, op1=alu.add, accum_out=se_c[:],
                    )
                else:
                    nc.vector.tensor_tensor(out=e_c, in0=t_c, in1=tm_c,
                                            op=alu.mult)
                    sp_c = work.tile([BL, 1], F32, tag="sep")
                    nc.vector.tensor_reduce(out=sp_c, in_=e_c,
                                            axis=mybir.AxisListType.X, op=alu.add)
                    if se_prev is None:
                        se_c = sp_c
                    else:
                        nc.vector.tensor_tensor(out=se_c, in0=sp_c,
                                                in1=se_prev, op=alu.add)
                se_prev = se_c
                e_cs.append(e_c)
                if EEXP_ENG[c] == "act":
                    e_exp = work.tile([BL, tcsz * D], BF16, tag="eexp")
                    nc.scalar.copy(out=e_exp,
                                   in_=_bc(e_c[:], [[1, tcsz], [0, D]]))
                elif EEXP_ENG[c] == "dve":
                    e_exp = work.tile([BL, tcsz * D], BF16, tag="eexp")
                    nc.vector.tensor_copy(out=e_exp,
                                          in_=_bc(e_c[:], [[1, tcsz], [0, D]]))
                else:
                    e_exp = None
                e_exps.append(e_exp)

            # ---- per t-chunk, pass 2: prod = k*e, halving tree over t -----
            abar_parts = []
            for c, tcsz in enumerate(TCS):
                toff = TOFF[c]
                lo, hi = toff * D, (toff + tcsz) * D
                prod_c = work.tile([BL, tcsz * D], BF16, tag="prod")
                if e_exps[c] is not None:
                    nc.vector.tensor_tensor(
                        out=prod_c, in0=k_bf[:, lo:hi], in1=e_exps[c],
                        op=alu.mult,
                    )
                else:
                    nc.vector.tensor_tensor(
                        out=prod_c, in0=k_bf[:, lo:hi],
                        in1=_bc(e_cs[c][:], [[1, tcsz], [0, D]]), op=alu.mult,
                    )
                # halving tree inside one scratch tile (levels at offsets)
                scratch = work.tile([BL, tcsz * D], BF16, tag="tree")
                cur, off, w = prod_c, 0, tcsz * D
                while w > 2 * D and (w // D) % 2 == 0:
                    w //= 2
                    nxt = scratch[:, off:off + w]
                    eng = (nc.gpsimd if (USE_POOL_COMPUTE and w <= 512 and c < 3)
                           else nc.vector)
                    eng.tensor_tensor(out=nxt, in0=cur[:, :w],
                                      in1=cur[:, w:2 * w], op=alu.add)
                    cur = nxt
                    off += w
                abar_c = work.tile([BL, D], F32, tag="abar_c")
                if w == 2 * D:
                    nc.vector.tensor_tensor(out=abar_c, in0=cur[:, :D],
                                            in1=cur[:, D:2 * D], op=alu.add)
                else:
                    nc.vector.tensor_reduce(
                        out=abar_c, in_=_bc(cur[:], [[1, D], [D, w // D]]),
                        axis=mybir.AxisListType.X, op=alu.add,
                    )
                abar_parts.append(abar_c)

            ab01 = work.tile([BL, D], F32, tag="ab01")
            nc.vector.tensor_tensor(out=ab01, in0=abar_parts[0],
                                    in1=abar_parts[1], op=alu.add)
            ab23 = work.tile([BL, D], F32, tag="ab23")
            nc.vector.tensor_tensor(out=ab23, in0=abar_parts[2],
                                    in1=abar_parts[3], op=alu.add)
            ab014 = work.tile([BL, D], F32, tag="ab014")
            nc.vector.tensor_tensor(out=ab014, in0=ab01,
                                    in1=abar_parts[4], op=alu.add)
            rs = small.tile([BL, 1], F32)
            nc.vector.reciprocal(rs, se_prev)
            abar = small.tile([BL, D], F32)
            nc.vector.tensor_tensor(out=abar, in0=ab014, in1=ab23, op=alu.add)
            nc.vector.tensor_scalar(
                out=abar, in0=abar, scalar1=rs[:], scalar2=None, op0=alu.mult
            )

            # ---- out = abar @ W_value  (f32 PE chain) ---------------------
            abarT_p = psum2.tile([D, BL], F32, tag="ps_t")
            nc.tensor.transpose(abarT_p, abar, identf)
            abarT = small.tile([D, BL], F32)
            nc.scalar.copy(out=abarT, in_=abarT_p)

            o_p = psum2.tile([BL, D], F32, tag="ps_v")
            nc.tensor.matmul(o_p, abarT, wv_s, start=True, stop=True)
            o_s = small.tile([BL, D], F32)
            nc.scalar.copy(out=o_s, in_=o_p)
            nc.sync.dma_start(out=out_d[:, :], in_=o_s)

    nc.finalize()
    return nc


_NC_CACHE = {}


def _get_nc():
    if "nc" not in _NC_CACHE:
        _NC_CACHE["nc"] = build_nc()
    return _NC_CACHE["nc"]


def make_in_maps(q, k, kes_length, field_strengths, bias, W_query, W_key, W_value):
    f32 = np.float32
    maps = []
    for c in range(NCORES):
        sl = slice(c * BL, (c + 1) * BL)
        maps.append({
            "q": np.ascontiguousarray(q[sl].reshape(BL, F * D), dtype=f32),
            "k": np.ascontiguousarray(k[sl].reshape(BL, T * D), dtype=f32),
            "kes": np.ascontiguousarray(kes_length[sl].reshape(BL, 1), dtype=np.int32),
            "fs": np.ascontiguousarray(field_strengths.reshape(F, 1), dtype=f32),
            "bias": np.ascontiguousarray(bias.reshape(1, 1), dtype=f32),
            "wq": np.ascontiguousarray(W_query, dtype=f32),
            "wk": np.ascontiguousarray(W_key, dtype=f32),
            "wv": np.ascontiguousarray(W_value, dtype=f32),
        })
    return maps


def kernel(q, k, v, kes_length, field_strengths, bias, W_query, W_key, W_value,
           **_unused):
    nc = _get_nc()
    in_maps = make_in_maps(np.asarray(q), np.asarray(k), np.asarray(kes_length),
                           np.asarray(field_strengths), np.asarray(bias),
                           np.asarray(W_query), np.asarray(W_key),
                           np.asarray(W_value))
    res = run_bass_kernel_spmd(nc, in_maps, list(range(NCORES)))
    out = np.concatenate([res.results[c]["out"] for c in range(NCORES)], axis=0)
    return out.reshape(B, 1, D).astype(np.float32)
